# revision 39
# baseline (speedup 1.0000x reference)
"""Trainium2 8-core attention kernel for nn_Attention_8409545965959.

Reference computation (B=4, N=2048, C=1024, H=16 heads, Dh=64):
    qkv = x @ Wqkv; q,k,v per head
    att = softmax(where(mask>0, -1e7, q @ k^T / sqrt(Dh)))
    out = (att @ v) @ Wproj + bproj

Masked keys contribute exactly zero to the softmax (exp underflows to 0
in f32), so K/V are compacted host-side to the unmasked tokens of each
batch, padded to a multiple of 128 (padded positions re-masked on device
via the exp bias). This is an exact reformulation that shrinks the
attention k-dimension from 2048 to ~1152.

Sharding: tensor-parallel on heads (2 heads/core, column-parallel Wqkv).
Per (batch, 512-row q-block) the normalized attention output is
resharded with a small AllToAll ([8,128,64] shards: core c ends up
owning q rows qb*512 + c*64 + [0:64] of every q-block), and each core
computes full output rows (row-parallel proj over its 1024-row slice).
Final gather is host-side stitching.

On-device dataflow (per core, heads h0=2c, h1=2c+1):
  - activations kept transposed: qT/kT [128ch, n] from Wq/Wk-stationary
    matmuls vs host-transposed x^T; v in normal layout with a ones
    column appended per head: vt [128, kc, head, 65] (col 64 == 1.0).
  - S^T[k,q] per head via row-group-packed matmul pairs (K=Dh=64,
    tile_position (0,0)/(64,0)), both heads' scores in one PSUM tile
    [128, 1024].
  - softmax: exp via ScalarE activation (scale=1/sqrt(Dh), per-partition
    bias = -30000 on masked/padded k rows -> exact zeros), E^T in bf16.
    The scalar engine is the rate limiter of the attention inner loop
    (~1us per [128,1024] exp); everything else hides behind it.
  - O^T and the softmax denominators accumulate in ONE matmul per head
    per chunk: lhsT = [v_h | 1] (M=65) so PSUM row 64 is D_h. No
    separate denominator matmuls.
  - normalization: 1/D via reciprocal_approx_fast, per-head broadcast to
    64 partitions with K=2 bf16 selector matmuls, O^T * (1/D) on VectorE
    -> bf16, sliced into 64-q-column shards for the AllToAll.
  - one AllToAll per (batch, q-block) [8, 128, 64] fires as soon as that
    q-block is normalized; proj for batch b runs as filler during batch
    b+1, so only the last q-block's collective + proj(3) sit in the
    tail. Wproj-stationary proj produces out^T [1024, 256] per batch
    (+bias).

To keep the PE dense the emission order interleaves the next batch's
QKV matmuls (and the previous batch's proj) into the attention inner
loop as independent filler work. A small startup AllToAll absorbs
inter-core launch stagger off the critical path. Batch 0's K/V/Q work
is mostly woven into its own attention steps so the first scores fire
~12us in (DMA-limited on wq/wk + kb block 0 + xb q-block 0).

kernel(**inputs) accepts the full unsharded inputs and returns the full
[4, 2048, 1024] float32 output.
"""

import sys
import types

import numpy as np
import ml_dtypes

# If a caller enables BASS_TRACE without the axon NTFF profiling hook
# installed, concourse's trace path would fail importing
# antenv.axon_hooks. Provide a no-op fallback (never overrides a real
# module) so tracing degrades gracefully instead of crashing.
try:
    import antenv.axon_hooks  # noqa: F401
except ImportError:
    try:
        import antenv

        _ah = types.ModuleType("antenv.axon_hooks")
        _ah._hook = None
        _ah.set_axon_ntff_profile_hook = lambda h: setattr(_ah, "_hook", h)
        _ah.get_axon_ntff_profile_hook = lambda: _ah._hook
        sys.modules["antenv.axon_hooks"] = _ah
        antenv.axon_hooks = _ah
    except ImportError:
        pass

import concourse.bass as bass
import concourse.mybir as mybir
import concourse.tile as tile
from concourse import bacc
from concourse.bass_utils import run_bass_kernel_spmd

B = 4
N = 2048
C = 1024
H = 16
NCORES = 8
DH = C // H            # 64
HPC = H // NCORES      # 2 heads per core -> 128 channels/core
CPC = HPC * DH         # 128
ROWS = B * N           # 8192
QB = 512               # q block (one PSUM bank of f32)
QS = QB // NCORES      # 64 q columns per a2a shard
KCH = 128              # k chunk (partitions)
NQB = N // QB          # 4
CC = C // 128          # 8 contraction chunks
SCALE = DH ** -0.5     # 0.125
MASK_BIAS = -30000.0

DT = mybir.dt.float32
BF = mybir.dt.bfloat16
NPBF = ml_dtypes.bfloat16

_CACHE: dict = {}
LAST_RESULTS = None
DEBUG_DUMP = False


def _build(nkcs):
    """nkcs[b] = number of 128-row k-chunks after compaction per batch."""
    nkc = max(nkcs)          # buffer/layout stride
    nk = nkc * KCH
    nc = bacc.Bacc("TRN2", target_bir_lowering=False, debug=False, num_devices=NCORES)

    xT = nc.dram_tensor("xT", [C, ROWS], BF, kind="ExternalInput")
    xTk = nc.dram_tensor("xTk", [C, B * nk], BF, kind="ExternalInput")
    wq = nc.dram_tensor("wq", [C, CPC], BF, kind="ExternalInput")
    wk = nc.dram_tensor("wk", [C, CPC], BF, kind="ExternalInput")
    wv = nc.dram_tensor("wv", [C, CPC], BF, kind="ExternalInput")
    wp = nc.dram_tensor("wp", [C, C], BF, kind="ExternalInput")
    bvec = nc.dram_tensor("bvec", [128, CC], DT, kind="ExternalInput")
    mb = nc.dram_tensor("mb", [128, B * nkc], DT, kind="ExternalInput")
    sel2 = nc.dram_tensor("sel2", [2, 128], BF, kind="ExternalInput")
    out_ext = nc.dram_tensor("out", [C, B * 4 * QS], DT, kind="ExternalOutput")
    dbg = (
        nc.dram_tensor("dbg", [2 * (DH + 1) + 4, QB], DT, kind="ExternalOutput")
        if DEBUG_DUMP
        else None
    )
    dbg2 = (
        nc.dram_tensor("dbg2", [2 * DH, QB], DT, kind="ExternalOutput")
        if DEBUG_DUMP
        else None
    )
    dbg3 = (
        nc.dram_tensor("dbg3", [2 * DH, QB], BF, kind="ExternalOutput")
        if DEBUG_DUMP
        else None
    )
    dbg4 = (
        nc.dram_tensor("dbg4", [NCORES * 128, QS], BF, kind="ExternalOutput")
        if DEBUG_DUMP
        else None
    )
    dbg5 = (
        nc.dram_tensor("dbg5", [128, CC * NQB * QS], BF, kind="ExternalOutput")
        if DEBUG_DUMP
        else None
    )

    # k blocks for the K^T qkv matmuls (moving dim <= 512), per batch.
    # Batch 0 gets a small first block so the first S matmul only waits
    # on one k-chunk's worth of data/compute at startup.
    def mk_kblocks(n, first_small=False):
        blocks = []
        pos = 0
        if first_small and n > KCH:
            blocks.append((0, KCH))
            pos = KCH
        while pos < n:
            w = min(QB, n - pos)
            blocks.append((pos, w))
            pos += w
        return blocks

    kblocks_b = [
        mk_kblocks(nkcs[b] * KCH, first_small=(b == 0)) for b in range(B)
    ]

    with tile.TileContext(nc) as tc:
        with (
            tc.tile_pool(name="consts", bufs=1) as consts,
            tc.tile_pool(name="xpool", bufs=2) as xpool,
            tc.tile_pool(name="kpool", bufs=2) as kpool,
            tc.tile_pool(name="qkpool", bufs=2) as qkpool,
            tc.tile_pool(name="vpool", bufs=2) as vpool,
            tc.tile_pool(name="epool", bufs=6) as epool,
            tc.tile_pool(name="npool", bufs=2) as npool,
            tc.tile_pool(name="opool", bufs=2) as opool,
            tc.tile_pool(name="dram", bufs=1, space="DRAM") as dram,
            tc.tile_pool(name="s_ps", bufs=2, space="PSUM") as s_ps,
            tc.tile_pool(name="o_ps", bufs=1, space="PSUM") as o_ps,
            tc.tile_pool(name="aux_ps", bufs=2, space="PSUM") as aux_ps,
        ):
            # ---- persistent constants / weights (wp is loaded late: it is
            # first needed by proj(0) during batch 1's attention)
            wq_sb = consts.tile([128, CC, CPC], BF)
            wk_sb = consts.tile([128, CC, CPC], BF)
            wv_sb = consts.tile([128, CC, CPC], BF)
            wp_sb = consts.tile([128, CC, C], BF)
            bias_sb = consts.tile([128, CC], DT)
            mb_sb = consts.tile([128, B * nkc], DT)
            sel2_sb = consts.tile([2, 128], BF)
            nc.sync.dma_start(wq_sb[:], wq.rearrange("(cc p) m -> p cc m", p=128))
            nc.sync.dma_start(wk_sb[:], wk.rearrange("(cc p) m -> p cc m", p=128))
            nc.sync.dma_start(wv_sb[:], wv.rearrange("(cc p) m -> p cc m", p=128))
            nc.sync.dma_start(bias_sb[:], bvec[:])
            nc.sync.dma_start(mb_sb[:], mb[:])
            nc.sync.dma_start(sel2_sb[:], sel2[:])

            # AllToAll bounce buffers. Shard j of batch b carries q columns
            # j*64..(j+1)*64 of each q-block (so core j owns those rows).
            # Batches 0-2 exchange once per batch (bandwidth-efficient);
            # batch 3 is split [qb0-2 | qb3] so only a 64-column piece of
            # the exchange sits in the tail.
            def a2a_pair(name, cols):
                i = dram.tile([NCORES, 128, cols], BF, name=f"{name}i", tag=f"{name}i")
                o = dram.tile([NCORES, 128, cols], BF, name=f"{name}o", tag=f"{name}o")
                return i, o

            a2a = {}
            for b in range(B - 1):
                a2a[b] = a2a_pair(f"a2a{b}", NQB * QS)
            a2a["3a"] = a2a_pair("a2a3a", 3 * QS)
            a2a["3b"] = a2a_pair("a2a3b", QS)

            # startup alignment: absorb inter-core launch stagger on the
            # collective engine before real barriers sit on the critical path
            align_in = dram.tile([2, 4], BF, name="align_in", tag="align_in")
            align_out = dram.tile([2, 4], BF, name="align_out", tag="align_out")
            nc.sync.dma_start(align_in[:], sel2[0:2, 0:4])
            nc.gpsimd.collective_compute(
                "AllToAll",
                mybir.AluOpType.bypass,
                ins=[align_in.opt()],
                outs=[align_out.opt()],
                replica_groups=[list(range(NCORES))],
            )

            def emit_collective(key):
                def emit():
                    nc.gpsimd.collective_compute(
                        "AllToAll",
                        mybir.AluOpType.bypass,
                        ins=[a2a[key][0].opt()],
                        outs=[a2a[key][1].opt()],
                        replica_groups=[list(range(NCORES))],
                    )

                return emit

            xb_tiles = {}
            kb_tiles = {}
            qkv_state = {}

            def emit_xb_load(b, split_first=False):
                nk_b = nkcs[b] * KCH
                kblocks = kblocks_b[b]
                xb = xpool.tile([128, CC, N], BF, name=f"xb{b}", tag="xb")
                xs = xT[:, b * N:(b + 1) * N].rearrange("(cc p) n -> p cc n", p=128)
                kb = kpool.tile([128, CC, nk], BF, name=f"kb{b}", tag="kb")
                ks = xTk[:, b * nk:b * nk + nk_b].rearrange(
                    "(cc p) n -> p cc n", p=128
                )
                xb_tiles[b] = xb
                kb_tiles[b] = kb
                if not split_first:
                    for cc in range(CC):
                        nc.sync.dma_start(xb[:, cc, :], xs[:, cc, :])
                        nc.sync.dma_start(kb[:, cc, 0:nk_b], ks[:, cc, :])
                    return
                # batch 0: order DMAs so the first attention step's inputs
                # land first: kb block 0, xb q-block 0, rest of kb, rest of
                # xb. (Slicing matches the consumers' read regions.)
                p0, w0 = kblocks[0]
                for cc in range(CC):
                    nc.sync.dma_start(kb[:, cc, p0:p0 + w0], ks[:, cc, p0:p0 + w0])
                for cc in range(CC):
                    nc.sync.dma_start(xb[:, cc, 0:QB], xs[:, cc, 0:QB])
                for pos, w in kblocks[1:]:
                    for cc in range(CC):
                        nc.sync.dma_start(
                            kb[:, cc, pos:pos + w], ks[:, cc, pos:pos + w]
                        )
                for rb in range(1, NQB):
                    for cc in range(CC):
                        nc.sync.dma_start(
                            xb[:, cc, rb * QB:(rb + 1) * QB],
                            xs[:, cc, rb * QB:(rb + 1) * QB],
                        )

            def qkv_units(b):
                """Independent emission units for batch b's QKV (filler work).

                Returns (units, q_units, k_units, v_units) so batch 0 can
                hand-place them; for other batches `units` is spread evenly.
                """
                xb = xb_tiles[b]
                kb = kb_tiles[b]
                nkc_b = nkcs[b]
                qT = qkpool.tile([128, N], BF, name=f"qT{b}", tag="qT")
                kT = qkpool.tile([128, nk], BF, name=f"kT{b}", tag="kT")
                vt = vpool.tile([128, nkc, HPC, DH + 1], BF, name=f"vt{b}", tag="vt")
                qkv_state[b] = (qT, kT, vt)

                def ones_unit():
                    nc.vector.memset(vt[:, :, :, DH:DH + 1], 1.0)

                def q_unit(rb):
                    def emit():
                        ps = aux_ps.tile([128, QB], DT, name=f"psq{b}_{rb}", tag="aux")
                        for cc in range(CC):
                            nc.tensor.matmul(
                                ps[:],
                                wq_sb[:, cc, :],
                                xb[:, cc, rb * QB:(rb + 1) * QB],
                                start=cc == 0,
                                stop=cc == CC - 1,
                            )
                        nc.vector.tensor_copy(qT[:, rb * QB:(rb + 1) * QB], ps[:])

                    return emit

                def k_unit(pos, w):
                    def emit():
                        ps = aux_ps.tile([128, QB], DT, name=f"psk{b}_{pos}", tag="aux")
                        for cc in range(CC):
                            nc.tensor.matmul(
                                ps[:, 0:w],
                                wk_sb[:, cc, :],
                                kb[:, cc, pos:pos + w],
                                start=cc == 0,
                                stop=cc == CC - 1,
                            )
                        nc.vector.tensor_copy(kT[:, pos:pos + w], ps[:, 0:w])

                    return emit

                def v_unit(rc):
                    def emit():
                        ps = aux_ps.tile([128, QB], DT, name=f"psv{b}_{rc}", tag="aux")
                        for cc in range(CC):
                            nc.tensor.matmul(
                                ps[:, 0:CPC],
                                kb[:, cc, rc * KCH:(rc + 1) * KCH],
                                wv_sb[:, cc, :],
                                start=cc == 0,
                                stop=cc == CC - 1,
                            )
                        for h in range(HPC):
                            nc.vector.tensor_copy(
                                vt[:, rc, h, 0:DH], ps[:, h * DH:(h + 1) * DH]
                            )

                    return emit

                q_units = [q_unit(rb) for rb in range(NQB)]
                k_units = [k_unit(pos, w) for pos, w in kblocks_b[b]]
                v_units = [v_unit(rc) for rc in range(nkc_b)]
                units = [ones_unit] + q_units + k_units + v_units
                return units, q_units, k_units, v_units, ones_unit

            def attention_steps(b, carried=None):
                """One closure per (qb, kc) plus the per-qb normalization.

                `carried` is a list of closures from the previous batch
                (deferred norm_b + its collective) to weave in early.
                """
                qT, kT, vt = qkv_state[b]
                nkc_b = nkcs[b]
                kc_lists = [[] for _ in range(NQB)]
                norm_pairs = []
                for qb in range(NQB):
                    steps = kc_lists[qb]
                    o0 = o_ps.tile([DH + 1, QB], DT, name=f"o0_{b}_{qb}", tag="o0")
                    o1 = o_ps.tile([DH + 1, QB], DT, name=f"o1_{b}_{qb}", tag="o1")

                    for kc in range(nkc_b):
                        def kc_step(qb=qb, kc=kc, o0=o0, o1=o1):
                            s2 = s_ps.tile(
                                [128, 2 * QB], DT, name=f"s{b}_{qb}_{kc}", tag="s"
                            )
                            nc.tensor.matmul(
                                s2[:, 0:QB],
                                kT[0:DH, kc * KCH:(kc + 1) * KCH],
                                qT[0:DH, qb * QB:(qb + 1) * QB],
                                start=True,
                                stop=True,
                                tile_position=(0, 0),
                            )
                            nc.tensor.matmul(
                                s2[:, QB:2 * QB],
                                kT[DH:2 * DH, kc * KCH:(kc + 1) * KCH],
                                qT[DH:2 * DH, qb * QB:(qb + 1) * QB],
                                start=True,
                                stop=True,
                                tile_position=(64, 0),
                            )
                            e2 = epool.tile(
                                [128, 2 * QB], BF, name=f"e{b}_{qb}_{kc}", tag="e"
                            )
                            mcol = b * nkc + kc
                            nc.scalar.activation(
                                e2[:],
                                s2[:],
                                mybir.ActivationFunctionType.Exp,
                                bias=mb_sb[:, mcol:mcol + 1],
                                scale=SCALE,
                            )
                            st = kc == 0
                            sp = kc == nkc_b - 1
                            # O^T plus denominator in one matmul per head:
                            # vt[...,64] == 1.0 so PSUM row 64 is D_h.
                            nc.tensor.matmul(
                                o0[:],
                                vt[:, kc, 0, :],
                                e2[:, 0:QB],
                                start=st,
                                stop=sp,
                            )
                            nc.tensor.matmul(
                                o1[:],
                                vt[:, kc, 1, :],
                                e2[:, QB:2 * QB],
                                start=st,
                                stop=sp,
                            )

                        steps.append(kc_step)

                    state = {}

                    def norm_a(qb=qb, o0=o0, o1=o1, state=state):
                        # free the PSUM accumulators immediately
                        osb0 = opool.tile(
                            [DH + 1, QB], DT, name=f"osb0_{b}_{qb}", tag="osb0"
                        )
                        osb1 = opool.tile(
                            [DH + 1, QB], DT, name=f"osb1_{b}_{qb}", tag="osb1"
                        )
                        nc.vector.tensor_copy(osb0[:], o0[:])
                        nc.vector.tensor_copy(osb1[:], o1[:])
                        if DEBUG_DUMP and b == 0 and qb == 0:
                            nc.sync.dma_start(dbg[0:DH + 1, :], osb0[:])
                            nc.sync.dma_start(dbg[DH + 1:2 * (DH + 1), :], osb1[:])
                        state["osb0"] = osb0
                        state["osb1"] = osb1

                    def norm_b(qb=qb, state=state):
                        # deferred: the dd-DMA/reciprocal chain latency hides
                        # behind the next q-block's attention matmuls
                        osb0 = state["osb0"]
                        osb1 = state["osb1"]
                        dd = npool.tile([2, QB], DT, name=f"dd{b}_{qb}", tag="dd")
                        # batches 0-2: gpsimd DMA queue keeps the norm path
                        # off the Sync queue (bulk-load head-of-line
                        # blocking). Batch 3: Sync is empty (no loads, no
                        # fillers) and issues ~40% faster than gpsimd, so the
                        # tail-critical chain goes there.
                        eng = nc.sync if b == B - 1 else nc.gpsimd
                        eng.dma_start(dd[0:1, :], osb0[DH:DH + 1, :])
                        eng.dma_start(dd[1:2, :], osb1[DH:DH + 1, :])
                        dr = npool.tile([2, QB], DT, name=f"dr{b}_{qb}", tag="dr")
                        nc.vector.reciprocal_approx_fast(dr[:], dd[:])
                        drbf = npool.tile([2, QB], BF, name=f"drbf{b}_{qb}", tag="drbf")
                        nc.vector.tensor_copy(drbf[:], dr[:])
                        if DEBUG_DUMP and b == 0 and qb == 0:
                            base = 2 * (DH + 1)
                            nc.sync.dma_start(dbg[base:base + 2, :], dd[:])
                            nc.sync.dma_start(dbg[base + 2:base + 4, :], dr[:])
                        # per-head broadcast of 1/D to 64 partitions
                        drA = aux_ps.tile([DH, QB], DT, name=f"drA{b}_{qb}", tag="aux")
                        drB = aux_ps.tile([DH, QB], DT, name=f"drB{b}_{qb}", tag="aux")
                        nc.tensor.matmul(
                            drA[:], sel2_sb[:, 0:DH], drbf[:], start=True, stop=True
                        )
                        nc.tensor.matmul(
                            drB[:], sel2_sb[:, DH:2 * DH], drbf[:], start=True, stop=True
                        )
                        of0 = opool.tile([DH, QB], BF, name=f"of0_{b}_{qb}", tag="of0")
                        of1 = opool.tile([DH, QB], BF, name=f"of1_{b}_{qb}", tag="of1")
                        nc.vector.tensor_mul(of0[:], osb0[0:DH, :], drA[:])
                        nc.vector.tensor_mul(of1[:], osb1[0:DH, :], drB[:])
                        if b < B - 1:
                            dst, col = a2a[b][0], qb * QS
                        elif qb < 3:
                            dst, col = a2a["3a"][0], qb * QS
                        else:
                            dst, col = a2a["3b"][0], 0
                        eng.dma_start(
                            dst[:, 0:DH, col:col + QS].rearrange("s p j -> p s j"),
                            of0.rearrange("p (s j) -> p s j", s=NCORES),
                        )
                        eng.dma_start(
                            dst[:, DH:2 * DH, col:col + QS].rearrange("s p j -> p s j"),
                            of1.rearrange("p (s j) -> p s j", s=NCORES),
                        )

                    norm_pairs.append((norm_a, norm_b))
                # weave: kc-steps of qb, then norm_a(qb); norm_b(qb) lands
                # after the first 2 kc-steps of qb+1. A batch's collective
                # fires right after its last contributing norm_b. The tail
                # [norm_b, collective] is returned so the caller can weave
                # it into the NEXT batch (or flush it at the end).
                woven = []
                pending = list(carried) if carried else []
                for qb in range(NQB):
                    for i in range(nkc_b):
                        woven.append(kc_lists[qb][i])
                        if i == 1 and pending:
                            woven.extend(pending)
                            pending = []
                    na, nb = norm_pairs[qb]
                    woven.append(na)
                    pending = [nb]
                    if b == B - 1 and qb == 2:
                        pending.append(emit_collective("3a"))
                    elif qb == NQB - 1:
                        pending.append(
                            emit_collective("3b" if b == B - 1 else b)
                        )
                return woven, pending

            def proj_units(grp, part=None):
                """Projection for batch `grp`. part=None: whole batch
                (grp < 3). For grp 3: part='a' covers q-blocks 0-2 (after
                the 3a exchange), part='b' the last 64 columns."""
                units = []
                q0, q1 = 0, NQB          # qb range this call covers
                if part == "a":
                    q1 = 3
                elif part == "b":
                    q0 = 3
                ncols = (q1 - q0) * QS

                def load_unit():
                    if part != "b":
                        ofull = qkpool.tile(
                            [128, CC, NQB, QS], BF, name=f"ofull{grp}", tag="ofull"
                        )
                        qkv_state[f"ofull{grp}"] = ofull
                    ofull = qkv_state[f"ofull{grp}"]
                    src = a2a[grp if part is None else ("3a" if part == "a" else "3b")][1]
                    nc.gpsimd.dma_start(
                        ofull[:, :, q0:q1, :],
                        src.rearrange("i p (q j) -> p i q j", q=q1 - q0),
                    )

                units.append(load_unit)

                # part 'b' (the very tail): collect all 8 oc outputs in one
                # tile and ship a single DMA — 8 serial DMA issues would sit
                # directly on the critical path.
                fo_all = [None]
                if part == "b":
                    def fo_alloc():
                        fo_all[0] = npool.tile(
                            [128, CC, ncols], DT, name=f"foall{grp}", tag="foall"
                        )
                    units.insert(0, fo_alloc)

                def oc_unit(oc):
                    def emit():
                        ofull = qkv_state[f"ofull{grp}"]
                        pps = aux_ps.tile([128, QB], DT, name=f"pp{grp}_{oc}{part or ''}", tag="aux")
                        for cc in range(CC):
                            nc.tensor.matmul(
                                pps[:, 0:ncols],
                                wp_sb[:, cc, oc * 128:(oc + 1) * 128],
                                ofull[:, cc, q0:q1, :],
                                start=cc == 0,
                                stop=cc == CC - 1,
                            )
                        if part == "b":
                            nc.vector.tensor_scalar_add(
                                fo_all[0][:, oc, :], pps[:, 0:ncols],
                                bias_sb[:, oc:oc + 1],
                            )
                            return
                        fo = npool.tile(
                            [128, ncols], DT, name=f"fo{grp}_{oc}{part or ''}", tag="fo"
                        )
                        nc.vector.tensor_scalar_add(
                            fo[:], pps[:, 0:ncols], bias_sb[:, oc:oc + 1]
                        )
                        nc.sync.dma_start(
                            out_ext[
                                oc * 128:(oc + 1) * 128,
                                grp * NQB * QS + q0 * QS:
                                grp * NQB * QS + q0 * QS + ncols,
                            ],
                            fo[:],
                        )

                    return emit

                for oc in range(CC):
                    units.append(oc_unit(oc))
                if part == "b":
                    def final_dma():
                        nc.sync.dma_start(
                            out_ext[
                                :, grp * NQB * QS + q0 * QS:
                                grp * NQB * QS + q0 * QS + ncols,
                            ].rearrange("(oc p) j -> p oc j", p=128),
                            fo_all[0][:],
                        )
                    units.append(final_dma)
                return units

            def run_interleaved(steps, fillers, pinned=None):
                """Emit `steps` in order; after step i, emit pinned[i] (a
                list) if given, and spread `fillers` evenly across steps."""
                pinned = pinned or {}
                nf = len(fillers)
                ns = len(steps)
                fi = 0
                for i, s in enumerate(steps):
                    s()
                    for p in pinned.get(i, ()):  # batch-0 hand placement
                        p()
                    if fi < nf and (i + 1) * nf >= (fi + 1) * ns:
                        fillers[fi]()
                        fi += 1
                while fi < nf:
                    fillers[fi]()
                    fi += 1

            # ---- schedule:
            #  batch 0: emit only k0/q0/v0 before attention; the rest of its
            #    QKV is pinned to the first steps. Batch 1's loads+QKV start
            #    at step 12 (after batch 0's own DMAs have drained).
            #  batch b: fillers = QKV(b+1) + proj(b-1); per-qb collectives
            #    are woven right after each norm_b.
            #  tail: last norm_b + collective(3,3), then proj(3).
            emit_xb_load(0, split_first=True)
            units0, q_units0, k_units0, v_units0, ones0 = qkv_units(0)
            ones0()
            k_units0[0]()
            q_units0[0]()
            v_units0[0]()
            nc.sync.dma_start(wp_sb[:], wp.rearrange("(cc p) m -> p cc m", p=128))

            pin0 = {
                0: [k_units0[i] for i in range(1, len(k_units0))] + [v_units0[1]],
                1: [v_units0[2], v_units0[3]],
                2: [v_units0[4], v_units0[5]],
                3: [v_units0[6], v_units0[7]],
                4: [v_units0[rc] for rc in range(8, nkcs[0])] + [q_units0[1]],
                6: [q_units0[2]],
                8: [q_units0[3]],
            }

            carried = None
            for b in range(B):
                fillers = []
                pinned = None
                if b == 0:
                    pinned = dict(pin0)
                    emit_xb_load(1)
                    units1 = qkv_units(1)[0]
                    # hold batch 1's QKV until batch 0's loads have drained
                    nsteps = NQB * nkcs[0]
                    for j, u in enumerate(units1):
                        pinned.setdefault(
                            12 + (j * (nsteps - 14)) // len(units1), []
                        ).append(u)
                else:
                    if b < B - 1:
                        emit_xb_load(b + 1)
                        fillers.extend(qkv_units(b + 1)[0])
                    if b < B - 1:
                        # batch 3 runs lean: proj(2) fills the tail's
                        # collective-wait gap instead of stretching the
                        # PE-bound attention phase.
                        fillers.extend(proj_units(b - 1))
                steps, carried = attention_steps(b, carried)
                run_interleaved(steps, fillers, pinned)
            # tail: last norm + tiny 3b exchange; proj(2) fills the 3a
            # collective wait, then proj(3) in two pieces so only the last
            # 64 columns depend on the final exchange.
            for u in carried:
                u()
            for u in proj_units(B - 2):
                u()
            for u in proj_units(B - 1, part="a"):
                u()
            for u in proj_units(B - 1, part="b"):
                u()

    nc.compile()
    return nc


def _prep_inputs(x, Wqkv, Wproj, bproj, mask, nkcs):
    x = np.asarray(x, dtype=np.float32)
    Wqkv = np.asarray(Wqkv, dtype=np.float32)
    Wproj = np.asarray(Wproj, dtype=np.float32)
    bproj = np.asarray(bproj, dtype=np.float32)
    mask = np.asarray(mask)
    nkc = max(nkcs)
    nk = nkc * KCH

    x2 = x.reshape(ROWS, C)
    xT = np.ascontiguousarray(x2.T).astype(NPBF)
    # compacted K/V tokens: unmasked columns per batch, zero-padded to nk
    xTk = np.zeros((C, B * nk), dtype=NPBF)
    mbias = np.full((B, nk), np.float32(MASK_BIAS), dtype=np.float32)
    for b in range(B):
        idx = np.nonzero(mask[b] == 0)[0]
        cnt = len(idx)
        xTk[:, b * nk: b * nk + cnt] = xT[:, b * N + idx]
        mbias[b, :cnt] = 0.0
    mb_arr = np.ascontiguousarray(
        mbias.reshape(B, nkc, 128).transpose(2, 0, 1).reshape(128, B * nkc)
    ).astype(np.float32)

    wp_bf = Wproj.astype(NPBF)
    bias_r = np.ascontiguousarray(bproj.reshape(CC, 128).T).astype(np.float32)
    sel2 = np.zeros((2, 128), np.float32)
    sel2[0, 0:64] = 1.0
    sel2[1, 64:128] = 1.0
    sel2 = sel2.astype(NPBF)

    in_maps = []
    for c in range(NCORES):
        cols = slice(c * CPC, (c + 1) * CPC)
        in_maps.append(
            dict(
                xT=xT,
                xTk=xTk,
                wq=np.ascontiguousarray(Wqkv[:, cols]).astype(NPBF),
                wk=np.ascontiguousarray(Wqkv[:, C:][:, cols]).astype(NPBF),
                wv=np.ascontiguousarray(Wqkv[:, 2 * C:][:, cols]).astype(NPBF),
                wp=wp_bf,
                bvec=bias_r,
                mb=mb_arr,
                sel2=sel2,
            )
        )
    return in_maps


def kernel(x, Wqkv, Wproj, bproj, mask):
    global LAST_RESULTS
    mask = np.asarray(mask)
    counts = (mask == 0).sum(axis=1)
    nkcs = tuple(max(1, -(-int(c) // KCH)) for c in counts)
    if nkcs not in _CACHE:
        _CACHE[nkcs] = _build(nkcs)
    nc = _CACHE[nkcs]
    in_maps = _prep_inputs(x, Wqkv, Wproj, bproj, mask, nkcs)
    res = run_bass_kernel_spmd(nc, in_maps, list(range(NCORES)))
    LAST_RESULTS = res
    out = np.empty((ROWS, C), dtype=np.float32)
    for c in range(NCORES):
        oT = res.results[c]["out"]  # [1024 oc, B * 4 qb * 64 q] = final^T
        for b in range(B):
            for qb in range(NQB):
                rows = slice(
                    b * N + qb * QB + c * QS, b * N + qb * QB + (c + 1) * QS
                )
                out[rows, :] = oT[:, b * NQB * QS + qb * QS:
                                  b * NQB * QS + (qb + 1) * QS].T
    return out.reshape(B, N, C)


# revision 41
# speedup vs baseline: 1.0737x; 1.0737x over previous
"""Trainium2 8-core attention kernel for nn_Attention_8409545965959.

Reference computation (B=4, N=2048, C=1024, H=16 heads, Dh=64):
    qkv = x @ Wqkv; q,k,v per head
    att = softmax(where(mask>0, -1e7, q @ k^T / sqrt(Dh)))
    out = (att @ v) @ Wproj + bproj

Masked keys contribute exactly zero to the softmax (exp underflows to 0
in f32), so K/V are compacted host-side to the unmasked tokens of each
batch, padded to a multiple of 128 (padded positions re-masked on device
via the exp bias). This is an exact reformulation that shrinks the
attention k-dimension from 2048 to ~1152.

Sharding: tensor-parallel on heads (2 heads/core, column-parallel Wqkv).
The normalized attention output is resharded with one AllToAll per
batch (shard j carries q columns j*64..(j+1)*64 of each 512-row
q-block, so core c ends up owning q rows qb*512 + c*64 + [0:64]), and
each core computes full output rows (row-parallel proj over its
1024-row slice). The last batch's exchange is split [q-blocks 0-2 |
q-block 3] so only a 64-column piece sits in the tail, where proj(2)
fills the collective wait. Final gather is host-side stitching.

On-device dataflow (per core, heads h0=2c, h1=2c+1):
  - activations kept transposed: qT/kT [128ch, n] from Wq/Wk-stationary
    matmuls vs host-transposed x^T; v in normal layout with a ones
    column appended per head: vt [128, kc, head, 65] (col 64 == 1.0).
  - S^T[k,q] per head via row-group-packed matmul pairs (K=Dh=64,
    tile_position (0,0)/(64,0)), both heads' scores in one PSUM tile
    [128, 1024].
  - softmax: exp via ScalarE activation (scale=1/sqrt(Dh), per-partition
    bias = -30000 on masked/padded k rows -> exact zeros), E^T in bf16.
    The scalar engine is the rate limiter of the attention inner loop
    (~1us per [128,1024] exp); everything else hides behind it.
  - O^T and the softmax denominators accumulate in ONE matmul per head
    per chunk: lhsT = [v_h | 1] (M=65) so PSUM row 64 is D_h. No
    separate denominator matmuls.
  - normalization: 1/D via reciprocal_approx_fast, per-head broadcast to
    64 partitions with K=2 bf16 selector matmuls, O^T * (1/D) on VectorE
    -> bf16, sliced into 64-q-column shards for the AllToAll.
  - the per-batch AllToAll fires right after the batch's last q-block
    is normalized; proj for batch b runs as filler during batch b+1
    (b<2) or in the tail (b=2, overlapping the split last exchange).
    Wproj-stationary proj produces out^T [1024, 256] per batch (+bias).
    Norm-path DMAs ride the gpsimd queue (batches 0-2) to dodge Sync
    head-of-line blocking, and the empty Sync queue in batch 3.

To keep the PE dense the emission order interleaves the next batch's
QKV matmuls (and the previous batch's proj) into the attention inner
loop as independent filler work. A small startup AllToAll absorbs
inter-core launch stagger off the critical path. Batch 0's K/V/Q work
is mostly woven into its own attention steps so the first scores fire
~12us in (DMA-limited on wq/wk + kb block 0 + xb q-block 0).

kernel(**inputs) accepts the full unsharded inputs and returns the full
[4, 2048, 1024] float32 output.
"""

import sys
import types

import numpy as np
import ml_dtypes

# If a caller enables BASS_TRACE without the axon NTFF profiling hook
# installed, concourse's trace path would fail importing
# antenv.axon_hooks. Provide a no-op fallback (never overrides a real
# module) so tracing degrades gracefully instead of crashing.
try:
    import antenv.axon_hooks  # noqa: F401
except ImportError:
    try:
        import antenv

        _ah = types.ModuleType("antenv.axon_hooks")
        _ah._hook = None
        _ah.set_axon_ntff_profile_hook = lambda h: setattr(_ah, "_hook", h)
        _ah.get_axon_ntff_profile_hook = lambda: _ah._hook
        sys.modules["antenv.axon_hooks"] = _ah
        antenv.axon_hooks = _ah
    except ImportError:
        pass

import concourse.bass as bass
import concourse.mybir as mybir
import concourse.tile as tile
from concourse import bacc
from concourse.bass_utils import run_bass_kernel_spmd

B = 4
N = 2048
C = 1024
H = 16
NCORES = 8
DH = C // H            # 64
HPC = H // NCORES      # 2 heads per core -> 128 channels/core
CPC = HPC * DH         # 128
ROWS = B * N           # 8192
QB = 512               # q block (one PSUM bank of f32)
QS = QB // NCORES      # 64 q columns per a2a shard
KCH = 128              # k chunk (partitions)
NQB = N // QB          # 4
CC = C // 128          # 8 contraction chunks
SCALE = DH ** -0.5     # 0.125
MASK_BIAS = -30000.0

DT = mybir.dt.float32
BF = mybir.dt.bfloat16
NPBF = ml_dtypes.bfloat16

_CACHE: dict = {}
LAST_RESULTS = None
DEBUG_DUMP = False


def _build(nkcs):
    """nkcs[b] = number of 128-row k-chunks after compaction per batch."""
    nkc = max(nkcs)          # buffer/layout stride
    nk = nkc * KCH
    nc = bacc.Bacc("TRN2", target_bir_lowering=False, debug=False, num_devices=NCORES)

    xT = nc.dram_tensor("xT", [C, ROWS], BF, kind="ExternalInput")
    xTk = nc.dram_tensor("xTk", [C, B * nk], BF, kind="ExternalInput")
    wq = nc.dram_tensor("wq", [C, CPC], BF, kind="ExternalInput")
    wk = nc.dram_tensor("wk", [C, CPC], BF, kind="ExternalInput")
    wv = nc.dram_tensor("wv", [C, CPC], BF, kind="ExternalInput")
    wp = nc.dram_tensor("wp", [C, C], BF, kind="ExternalInput")
    bvec = nc.dram_tensor("bvec", [128, CC], DT, kind="ExternalInput")
    mb = nc.dram_tensor("mb", [128, B * nkc], DT, kind="ExternalInput")
    sel2 = nc.dram_tensor("sel2", [2, 128], BF, kind="ExternalInput")
    out_ext = nc.dram_tensor("out", [C, B * 4 * QS], DT, kind="ExternalOutput")
    dbg = (
        nc.dram_tensor("dbg", [2 * (DH + 1) + 4, QB], DT, kind="ExternalOutput")
        if DEBUG_DUMP
        else None
    )
    dbg2 = (
        nc.dram_tensor("dbg2", [2 * DH, QB], DT, kind="ExternalOutput")
        if DEBUG_DUMP
        else None
    )
    dbg3 = (
        nc.dram_tensor("dbg3", [2 * DH, QB], BF, kind="ExternalOutput")
        if DEBUG_DUMP
        else None
    )
    dbg4 = (
        nc.dram_tensor("dbg4", [NCORES * 128, QS], BF, kind="ExternalOutput")
        if DEBUG_DUMP
        else None
    )
    dbg5 = (
        nc.dram_tensor("dbg5", [128, CC * NQB * QS], BF, kind="ExternalOutput")
        if DEBUG_DUMP
        else None
    )

    # k blocks for the K^T qkv matmuls (moving dim <= 512), per batch.
    # Batch 0 gets a small first block so the first S matmul only waits
    # on one k-chunk's worth of data/compute at startup.
    def mk_kblocks(n, first_small=False):
        blocks = []
        pos = 0
        if first_small and n > KCH:
            blocks.append((0, KCH))
            pos = KCH
        while pos < n:
            w = min(QB, n - pos)
            blocks.append((pos, w))
            pos += w
        return blocks

    kblocks_b = [
        mk_kblocks(nkcs[b] * KCH, first_small=(b == 0)) for b in range(B)
    ]

    with tile.TileContext(nc) as tc:
        with (
            tc.tile_pool(name="consts", bufs=1) as consts,
            tc.tile_pool(name="xpool", bufs=2) as xpool,
            tc.tile_pool(name="kpool", bufs=2) as kpool,
            tc.tile_pool(name="qkpool", bufs=2) as qkpool,
            tc.tile_pool(name="vpool", bufs=2) as vpool,
            tc.tile_pool(name="epool", bufs=6) as epool,
            tc.tile_pool(name="npool", bufs=2) as npool,
            tc.tile_pool(name="opool", bufs=2) as opool,
            tc.tile_pool(name="dram", bufs=1, space="DRAM") as dram,
            tc.tile_pool(name="s_ps", bufs=2, space="PSUM") as s_ps,
            tc.tile_pool(name="o_ps", bufs=1, space="PSUM") as o_ps,
            tc.tile_pool(name="aux_ps", bufs=2, space="PSUM") as aux_ps,
        ):
            # ---- persistent constants / weights (wp is loaded late: it is
            # first needed by proj(0) during batch 1's attention)
            wq_sb = consts.tile([128, CC, CPC], BF)
            wk_sb = consts.tile([128, CC, CPC], BF)
            wv_sb = consts.tile([128, CC, CPC], BF)
            wp_sb = consts.tile([128, CC, C], BF)
            bias_sb = consts.tile([128, CC], DT)
            mb_sb = consts.tile([128, B * nkc], DT)
            sel2_sb = consts.tile([2, 128], BF)
            nc.sync.dma_start(wq_sb[:], wq.rearrange("(cc p) m -> p cc m", p=128))
            nc.sync.dma_start(wk_sb[:], wk.rearrange("(cc p) m -> p cc m", p=128))
            nc.sync.dma_start(wv_sb[:], wv.rearrange("(cc p) m -> p cc m", p=128))
            nc.sync.dma_start(bias_sb[:], bvec[:])
            nc.sync.dma_start(mb_sb[:], mb[:])
            nc.sync.dma_start(sel2_sb[:], sel2[:])

            # AllToAll bounce buffers. Shard j of batch b carries q columns
            # j*64..(j+1)*64 of each q-block (so core j owns those rows).
            # Batches 0-2 exchange once per batch (bandwidth-efficient);
            # batch 3 is split [qb0-2 | qb3] so only a 64-column piece of
            # the exchange sits in the tail.
            def a2a_pair(name, cols):
                i = dram.tile([NCORES, 128, cols], BF, name=f"{name}i", tag=f"{name}i")
                o = dram.tile([NCORES, 128, cols], BF, name=f"{name}o", tag=f"{name}o")
                return i, o

            a2a = {}
            for b in range(B - 1):
                a2a[b] = a2a_pair(f"a2a{b}", NQB * QS)
            a2a["3a"] = a2a_pair("a2a3a", 3 * QS)
            a2a["3b"] = a2a_pair("a2a3b", QS)

            # startup alignment: absorb inter-core launch stagger on the
            # collective engine before real barriers sit on the critical path
            align_in = dram.tile([2, 4], BF, name="align_in", tag="align_in")
            align_out = dram.tile([2, 4], BF, name="align_out", tag="align_out")
            nc.sync.dma_start(align_in[:], sel2[0:2, 0:4])
            nc.gpsimd.collective_compute(
                "AllToAll",
                mybir.AluOpType.bypass,
                ins=[align_in.opt()],
                outs=[align_out.opt()],
                replica_groups=[list(range(NCORES))],
            )

            def emit_collective(key):
                def emit():
                    nc.gpsimd.collective_compute(
                        "AllToAll",
                        mybir.AluOpType.bypass,
                        ins=[a2a[key][0].opt()],
                        outs=[a2a[key][1].opt()],
                        replica_groups=[list(range(NCORES))],
                    )

                return emit

            xb_tiles = {}
            kb_tiles = {}
            qkv_state = {}

            def emit_xb_load(b, split_first=False):
                nk_b = nkcs[b] * KCH
                kblocks = kblocks_b[b]
                xb = xpool.tile([128, CC, N], BF, name=f"xb{b}", tag="xb")
                xs = xT[:, b * N:(b + 1) * N].rearrange("(cc p) n -> p cc n", p=128)
                kb = kpool.tile([128, CC, nk], BF, name=f"kb{b}", tag="kb")
                ks = xTk[:, b * nk:b * nk + nk_b].rearrange(
                    "(cc p) n -> p cc n", p=128
                )
                xb_tiles[b] = xb
                kb_tiles[b] = kb
                if not split_first:
                    for cc in range(CC):
                        nc.sync.dma_start(xb[:, cc, :], xs[:, cc, :])
                        nc.sync.dma_start(kb[:, cc, 0:nk_b], ks[:, cc, :])
                    return
                # batch 0: order DMAs so the first attention step's inputs
                # land first: kb block 0, xb q-block 0, rest of kb, rest of
                # xb. (Slicing matches the consumers' read regions.)
                p0, w0 = kblocks[0]
                for cc in range(CC):
                    nc.sync.dma_start(kb[:, cc, p0:p0 + w0], ks[:, cc, p0:p0 + w0])
                for cc in range(CC):
                    nc.sync.dma_start(xb[:, cc, 0:QB], xs[:, cc, 0:QB])
                for pos, w in kblocks[1:]:
                    for cc in range(CC):
                        nc.sync.dma_start(
                            kb[:, cc, pos:pos + w], ks[:, cc, pos:pos + w]
                        )
                for rb in range(1, NQB):
                    for cc in range(CC):
                        nc.sync.dma_start(
                            xb[:, cc, rb * QB:(rb + 1) * QB],
                            xs[:, cc, rb * QB:(rb + 1) * QB],
                        )

            def qkv_units(b):
                """Independent emission units for batch b's QKV (filler work).

                Returns (units, q_units, k_units, v_units) so batch 0 can
                hand-place them; for other batches `units` is spread evenly.
                """
                xb = xb_tiles[b]
                kb = kb_tiles[b]
                nkc_b = nkcs[b]
                qT = qkpool.tile([128, N], BF, name=f"qT{b}", tag="qT")
                kT = qkpool.tile([128, nk], BF, name=f"kT{b}", tag="kT")
                vt = vpool.tile([128, nkc, HPC, DH + 1], BF, name=f"vt{b}", tag="vt")
                qkv_state[b] = (qT, kT, vt)

                def ones_unit():
                    nc.vector.memset(vt[:, :, :, DH:DH + 1], 1.0)

                def q_unit(rb):
                    def emit():
                        ps = aux_ps.tile([128, QB], DT, name=f"psq{b}_{rb}", tag="aux")
                        for cc in range(CC):
                            nc.tensor.matmul(
                                ps[:],
                                wq_sb[:, cc, :],
                                xb[:, cc, rb * QB:(rb + 1) * QB],
                                start=cc == 0,
                                stop=cc == CC - 1,
                            )
                        nc.vector.tensor_copy(qT[:, rb * QB:(rb + 1) * QB], ps[:])

                    return emit

                def k_unit(pos, w):
                    def emit():
                        ps = aux_ps.tile([128, QB], DT, name=f"psk{b}_{pos}", tag="aux")
                        for cc in range(CC):
                            nc.tensor.matmul(
                                ps[:, 0:w],
                                wk_sb[:, cc, :],
                                kb[:, cc, pos:pos + w],
                                start=cc == 0,
                                stop=cc == CC - 1,
                            )
                        nc.vector.tensor_copy(kT[:, pos:pos + w], ps[:, 0:w])

                    return emit

                def v_unit(rc):
                    def emit():
                        ps = aux_ps.tile([128, QB], DT, name=f"psv{b}_{rc}", tag="aux")
                        for cc in range(CC):
                            nc.tensor.matmul(
                                ps[:, 0:CPC],
                                kb[:, cc, rc * KCH:(rc + 1) * KCH],
                                wv_sb[:, cc, :],
                                start=cc == 0,
                                stop=cc == CC - 1,
                            )
                        for h in range(HPC):
                            nc.vector.tensor_copy(
                                vt[:, rc, h, 0:DH], ps[:, h * DH:(h + 1) * DH]
                            )

                    return emit

                q_units = [q_unit(rb) for rb in range(NQB)]
                k_units = [k_unit(pos, w) for pos, w in kblocks_b[b]]
                v_units = [v_unit(rc) for rc in range(nkc_b)]
                units = [ones_unit] + q_units + k_units + v_units
                return units, q_units, k_units, v_units, ones_unit

            def attention_steps(b, carried=None):
                """One closure per (qb, kc) plus the per-qb normalization.

                `carried` is a list of closures from the previous batch
                (deferred norm_b + its collective) to weave in early.
                """
                qT, kT, vt = qkv_state[b]
                nkc_b = nkcs[b]
                kc_lists = [[] for _ in range(NQB)]
                norm_pairs = []
                for qb in range(NQB):
                    steps = kc_lists[qb]
                    o0 = o_ps.tile([DH + 1, QB], DT, name=f"o0_{b}_{qb}", tag="o0")
                    o1 = o_ps.tile([DH + 1, QB], DT, name=f"o1_{b}_{qb}", tag="o1")

                    for kc in range(nkc_b):
                        def kc_step(qb=qb, kc=kc, o0=o0, o1=o1):
                            s2 = s_ps.tile(
                                [128, 2 * QB], DT, name=f"s{b}_{qb}_{kc}", tag="s"
                            )
                            nc.tensor.matmul(
                                s2[:, 0:QB],
                                kT[0:DH, kc * KCH:(kc + 1) * KCH],
                                qT[0:DH, qb * QB:(qb + 1) * QB],
                                start=True,
                                stop=True,
                                tile_position=(0, 0),
                            )
                            nc.tensor.matmul(
                                s2[:, QB:2 * QB],
                                kT[DH:2 * DH, kc * KCH:(kc + 1) * KCH],
                                qT[DH:2 * DH, qb * QB:(qb + 1) * QB],
                                start=True,
                                stop=True,
                                tile_position=(64, 0),
                            )
                            e2 = epool.tile(
                                [128, 2 * QB], BF, name=f"e{b}_{qb}_{kc}", tag="e"
                            )
                            mcol = b * nkc + kc
                            nc.scalar.activation(
                                e2[:],
                                s2[:],
                                mybir.ActivationFunctionType.Exp,
                                bias=mb_sb[:, mcol:mcol + 1],
                                scale=SCALE,
                            )
                            st = kc == 0
                            sp = kc == nkc_b - 1
                            # O^T plus denominator in one matmul per head:
                            # vt[...,64] == 1.0 so PSUM row 64 is D_h.
                            nc.tensor.matmul(
                                o0[:],
                                vt[:, kc, 0, :],
                                e2[:, 0:QB],
                                start=st,
                                stop=sp,
                            )
                            nc.tensor.matmul(
                                o1[:],
                                vt[:, kc, 1, :],
                                e2[:, QB:2 * QB],
                                start=st,
                                stop=sp,
                            )

                        steps.append(kc_step)

                    state = {}

                    def norm_a(qb=qb, o0=o0, o1=o1, state=state):
                        # free the PSUM accumulators immediately
                        osb0 = opool.tile(
                            [DH + 1, QB], DT, name=f"osb0_{b}_{qb}", tag="osb0"
                        )
                        osb1 = opool.tile(
                            [DH + 1, QB], DT, name=f"osb1_{b}_{qb}", tag="osb1"
                        )
                        nc.vector.tensor_copy(osb0[:], o0[:])
                        nc.vector.tensor_copy(osb1[:], o1[:])
                        if DEBUG_DUMP and b == 0 and qb == 0:
                            nc.sync.dma_start(dbg[0:DH + 1, :], osb0[:])
                            nc.sync.dma_start(dbg[DH + 1:2 * (DH + 1), :], osb1[:])
                        state["osb0"] = osb0
                        state["osb1"] = osb1

                    def norm_b(qb=qb, state=state):
                        # deferred: the dd-DMA/reciprocal chain latency hides
                        # behind the next q-block's attention matmuls
                        osb0 = state["osb0"]
                        osb1 = state["osb1"]
                        dd = npool.tile([2, QB], DT, name=f"dd{b}_{qb}", tag="dd")
                        # batches 0-2: gpsimd DMA queue keeps the norm path
                        # off the Sync queue (bulk-load head-of-line
                        # blocking). Batch 3: Sync is empty (no loads, no
                        # fillers) and issues ~40% faster than gpsimd, so the
                        # tail-critical chain goes there.
                        eng = nc.sync if b == B - 1 else nc.gpsimd
                        eng.dma_start(dd[0:1, :], osb0[DH:DH + 1, :])
                        eng.dma_start(dd[1:2, :], osb1[DH:DH + 1, :])
                        dr = npool.tile([2, QB], DT, name=f"dr{b}_{qb}", tag="dr")
                        nc.vector.reciprocal_approx_fast(dr[:], dd[:])
                        drbf = npool.tile([2, QB], BF, name=f"drbf{b}_{qb}", tag="drbf")
                        nc.vector.tensor_copy(drbf[:], dr[:])
                        if DEBUG_DUMP and b == 0 and qb == 0:
                            base = 2 * (DH + 1)
                            nc.sync.dma_start(dbg[base:base + 2, :], dd[:])
                            nc.sync.dma_start(dbg[base + 2:base + 4, :], dr[:])
                        # per-head broadcast of 1/D to 64 partitions
                        drA = aux_ps.tile([DH, QB], DT, name=f"drA{b}_{qb}", tag="aux")
                        drB = aux_ps.tile([DH, QB], DT, name=f"drB{b}_{qb}", tag="aux")
                        nc.tensor.matmul(
                            drA[:], sel2_sb[:, 0:DH], drbf[:], start=True, stop=True
                        )
                        nc.tensor.matmul(
                            drB[:], sel2_sb[:, DH:2 * DH], drbf[:], start=True, stop=True
                        )
                        of0 = opool.tile([DH, QB], BF, name=f"of0_{b}_{qb}", tag="of0")
                        of1 = opool.tile([DH, QB], BF, name=f"of1_{b}_{qb}", tag="of1")
                        nc.vector.tensor_mul(of0[:], osb0[0:DH, :], drA[:])
                        nc.vector.tensor_mul(of1[:], osb1[0:DH, :], drB[:])
                        if b < B - 1:
                            dst, col = a2a[b][0], qb * QS
                        elif qb < 3:
                            dst, col = a2a["3a"][0], qb * QS
                        else:
                            dst, col = a2a["3b"][0], 0
                        eng.dma_start(
                            dst[:, 0:DH, col:col + QS].rearrange("s p j -> p s j"),
                            of0.rearrange("p (s j) -> p s j", s=NCORES),
                        )
                        eng.dma_start(
                            dst[:, DH:2 * DH, col:col + QS].rearrange("s p j -> p s j"),
                            of1.rearrange("p (s j) -> p s j", s=NCORES),
                        )

                    norm_pairs.append((norm_a, norm_b))
                # weave: kc-steps of qb, then norm_a(qb); norm_b(qb) lands
                # after the first 2 kc-steps of qb+1. A batch's collective
                # fires right after its last contributing norm_b. The tail
                # [norm_b, collective] is returned so the caller can weave
                # it into the NEXT batch (or flush it at the end).
                woven = []
                pending = list(carried) if carried else []
                for qb in range(NQB):
                    for i in range(nkc_b):
                        woven.append(kc_lists[qb][i])
                        if i == 1 and pending:
                            woven.extend(pending)
                            pending = []
                    na, nb = norm_pairs[qb]
                    woven.append(na)
                    pending = [nb]
                    if b == B - 1 and qb == 2:
                        pending.append(emit_collective("3a"))
                    elif qb == NQB - 1:
                        pending.append(
                            emit_collective("3b" if b == B - 1 else b)
                        )
                return woven, pending

            def proj_units(grp, part=None):
                """Projection for batch `grp`. part=None: whole batch
                (grp < 3). For grp 3: part='a' covers q-blocks 0-2 (after
                the 3a exchange), part='b' the last 64 columns."""
                units = []
                q0, q1 = 0, NQB          # qb range this call covers
                if part == "a":
                    q1 = 3
                elif part == "b":
                    q0 = 3
                ncols = (q1 - q0) * QS

                def load_unit():
                    if part != "b":
                        ofull = qkpool.tile(
                            [128, CC, NQB, QS], BF, name=f"ofull{grp}", tag="ofull"
                        )
                        qkv_state[f"ofull{grp}"] = ofull
                    ofull = qkv_state[f"ofull{grp}"]
                    src = a2a[grp if part is None else ("3a" if part == "a" else "3b")][1]
                    nc.gpsimd.dma_start(
                        ofull[:, :, q0:q1, :],
                        src.rearrange("i p (q j) -> p i q j", q=q1 - q0),
                    )

                units.append(load_unit)

                # part 'b' (the very tail): collect all 8 oc outputs in one
                # tile and ship a single DMA — 8 serial DMA issues would sit
                # directly on the critical path.
                fo_all = [None]
                if part == "b":
                    def fo_alloc():
                        fo_all[0] = npool.tile(
                            [128, CC, ncols], DT, name=f"foall{grp}", tag="foall"
                        )
                    units.insert(0, fo_alloc)

                def oc_unit(oc):
                    def emit():
                        ofull = qkv_state[f"ofull{grp}"]
                        pps = aux_ps.tile([128, QB], DT, name=f"pp{grp}_{oc}{part or ''}", tag="aux")
                        for cc in range(CC):
                            nc.tensor.matmul(
                                pps[:, 0:ncols],
                                wp_sb[:, cc, oc * 128:(oc + 1) * 128],
                                ofull[:, cc, q0:q1, :],
                                start=cc == 0,
                                stop=cc == CC - 1,
                            )
                        if part == "b":
                            nc.vector.tensor_scalar_add(
                                fo_all[0][:, oc, :], pps[:, 0:ncols],
                                bias_sb[:, oc:oc + 1],
                            )
                            return
                        fo = npool.tile(
                            [128, ncols], DT, name=f"fo{grp}_{oc}{part or ''}", tag="fo"
                        )
                        nc.vector.tensor_scalar_add(
                            fo[:], pps[:, 0:ncols], bias_sb[:, oc:oc + 1]
                        )
                        nc.sync.dma_start(
                            out_ext[
                                oc * 128:(oc + 1) * 128,
                                grp * NQB * QS + q0 * QS:
                                grp * NQB * QS + q0 * QS + ncols,
                            ],
                            fo[:],
                        )

                    return emit

                for oc in range(CC):
                    units.append(oc_unit(oc))
                if part == "b":
                    def final_dma():
                        nc.sync.dma_start(
                            out_ext[
                                :, grp * NQB * QS + q0 * QS:
                                grp * NQB * QS + q0 * QS + ncols,
                            ].rearrange("(oc p) j -> p oc j", p=128),
                            fo_all[0][:],
                        )
                    units.append(final_dma)
                return units

            def run_interleaved(steps, fillers, pinned=None):
                """Emit `steps` in order; after step i, emit pinned[i] (a
                list) if given, and spread `fillers` evenly across steps."""
                pinned = pinned or {}
                nf = len(fillers)
                ns = len(steps)
                fi = 0
                for i, s in enumerate(steps):
                    s()
                    for p in pinned.get(i, ()):  # batch-0 hand placement
                        p()
                    if fi < nf and (i + 1) * nf >= (fi + 1) * ns:
                        fillers[fi]()
                        fi += 1
                while fi < nf:
                    fillers[fi]()
                    fi += 1

            # ---- schedule:
            #  batch 0: emit only k0/q0/v0 before attention; the rest of its
            #    QKV is pinned to the first steps. Batch 1's loads+QKV start
            #    at step 12 (after batch 0's own DMAs have drained).
            #  batch b: fillers = QKV(b+1) + proj(b-1); per-qb collectives
            #    are woven right after each norm_b.
            #  tail: last norm_b + collective(3,3), then proj(3).
            emit_xb_load(0, split_first=True)
            units0, q_units0, k_units0, v_units0, ones0 = qkv_units(0)
            ones0()
            k_units0[0]()
            q_units0[0]()
            v_units0[0]()
            nc.sync.dma_start(wp_sb[:], wp.rearrange("(cc p) m -> p cc m", p=128))

            pin0 = {
                0: [k_units0[i] for i in range(1, len(k_units0))] + [v_units0[1]],
                1: [v_units0[2], v_units0[3]],
                2: [v_units0[4], v_units0[5]],
                3: [v_units0[6], v_units0[7]],
                4: [v_units0[rc] for rc in range(8, nkcs[0])] + [q_units0[1]],
                6: [q_units0[2]],
                8: [q_units0[3]],
            }

            carried = None
            for b in range(B):
                fillers = []
                pinned = None
                if b == 0:
                    pinned = dict(pin0)
                    emit_xb_load(1)
                    units1 = qkv_units(1)[0]
                    # hold batch 1's QKV until batch 0's loads have drained
                    nsteps = NQB * nkcs[0]
                    for j, u in enumerate(units1):
                        pinned.setdefault(
                            12 + (j * (nsteps - 14)) // len(units1), []
                        ).append(u)
                else:
                    if b < B - 1:
                        emit_xb_load(b + 1)
                        fillers.extend(qkv_units(b + 1)[0])
                    if b < B - 1:
                        # batch 3 runs lean: proj(2) fills the tail's
                        # collective-wait gap instead of stretching the
                        # PE-bound attention phase.
                        fillers.extend(proj_units(b - 1))
                steps, carried = attention_steps(b, carried)
                run_interleaved(steps, fillers, pinned)
            # tail: last norm + tiny 3b exchange; proj(2) fills the 3a
            # collective wait, then proj(3) in two pieces so only the last
            # 64 columns depend on the final exchange.
            for u in carried:
                u()
            for u in proj_units(B - 2):
                u()
            for u in proj_units(B - 1, part="a"):
                u()
            for u in proj_units(B - 1, part="b"):
                u()

    nc.compile()
    return nc


def _prep_inputs(x, Wqkv, Wproj, bproj, mask, nkcs):
    x = np.asarray(x, dtype=np.float32)
    Wqkv = np.asarray(Wqkv, dtype=np.float32)
    Wproj = np.asarray(Wproj, dtype=np.float32)
    bproj = np.asarray(bproj, dtype=np.float32)
    mask = np.asarray(mask)
    nkc = max(nkcs)
    nk = nkc * KCH

    x2 = x.reshape(ROWS, C)
    xT = np.ascontiguousarray(x2.T).astype(NPBF)
    # compacted K/V tokens: unmasked columns per batch, zero-padded to nk
    xTk = np.zeros((C, B * nk), dtype=NPBF)
    mbias = np.full((B, nk), np.float32(MASK_BIAS), dtype=np.float32)
    for b in range(B):
        idx = np.nonzero(mask[b] == 0)[0]
        cnt = len(idx)
        xTk[:, b * nk: b * nk + cnt] = xT[:, b * N + idx]
        mbias[b, :cnt] = 0.0
    mb_arr = np.ascontiguousarray(
        mbias.reshape(B, nkc, 128).transpose(2, 0, 1).reshape(128, B * nkc)
    ).astype(np.float32)

    wp_bf = Wproj.astype(NPBF)
    bias_r = np.ascontiguousarray(bproj.reshape(CC, 128).T).astype(np.float32)
    sel2 = np.zeros((2, 128), np.float32)
    sel2[0, 0:64] = 1.0
    sel2[1, 64:128] = 1.0
    sel2 = sel2.astype(NPBF)

    in_maps = []
    for c in range(NCORES):
        cols = slice(c * CPC, (c + 1) * CPC)
        in_maps.append(
            dict(
                xT=xT,
                xTk=xTk,
                wq=np.ascontiguousarray(Wqkv[:, cols]).astype(NPBF),
                wk=np.ascontiguousarray(Wqkv[:, C:][:, cols]).astype(NPBF),
                wv=np.ascontiguousarray(Wqkv[:, 2 * C:][:, cols]).astype(NPBF),
                wp=wp_bf,
                bvec=bias_r,
                mb=mb_arr,
                sel2=sel2,
            )
        )
    return in_maps


def kernel(x, Wqkv, Wproj, bproj, mask):
    global LAST_RESULTS
    mask = np.asarray(mask)
    counts = (mask == 0).sum(axis=1)
    nkcs = tuple(max(1, -(-int(c) // KCH)) for c in counts)
    if nkcs not in _CACHE:
        _CACHE[nkcs] = _build(nkcs)
    nc = _CACHE[nkcs]
    in_maps = _prep_inputs(x, Wqkv, Wproj, bproj, mask, nkcs)
    res = run_bass_kernel_spmd(nc, in_maps, list(range(NCORES)))
    LAST_RESULTS = res
    out = np.empty((ROWS, C), dtype=np.float32)
    for c in range(NCORES):
        oT = res.results[c]["out"]  # [1024 oc, B * 4 qb * 64 q] = final^T
        for b in range(B):
            for qb in range(NQB):
                rows = slice(
                    b * N + qb * QB + c * QS, b * N + qb * QB + (c + 1) * QS
                )
                out[rows, :] = oT[:, b * NQB * QS + qb * QS:
                                  b * NQB * QS + (qb + 1) * QS].T
    return out.reshape(B, N, C)


# revision 47
# speedup vs baseline: 1.0790x; 1.0049x over previous
"""Trainium2 8-core attention kernel for nn_Attention_8409545965959.

Reference computation (B=4, N=2048, C=1024, H=16 heads, Dh=64):
    qkv = x @ Wqkv; q,k,v per head
    att = softmax(where(mask>0, -1e7, q @ k^T / sqrt(Dh)))
    out = (att @ v) @ Wproj + bproj

Masked keys contribute exactly zero to the softmax (exp underflows to 0
in f32), so K/V are compacted host-side to the unmasked tokens of each
batch, padded to a multiple of 128 (padded positions re-masked on device
via the exp bias). This is an exact reformulation that shrinks the
attention k-dimension from 2048 to ~1152.

Sharding: tensor-parallel on heads (2 heads/core, column-parallel Wqkv).
The normalized attention output is resharded with one AllToAll per
batch (shard j carries q columns j*64..(j+1)*64 of each 512-row
q-block, so core c ends up owning q rows qb*512 + c*64 + [0:64]), and
each core computes full output rows (row-parallel proj over its
1024-row slice). The last batch's exchange is split [q-blocks 0-2 |
q-block 3] so only a 64-column piece sits in the tail, where proj(2)
fills the collective wait. Final gather is host-side stitching.

On-device dataflow (per core, heads h0=2c, h1=2c+1):
  - activations kept transposed: qT/kT [128ch, n] from Wq/Wk-stationary
    matmuls vs host-transposed x^T; v in normal layout with a ones
    column appended per head: vt [128, kc, head, 65] (col 64 == 1.0).
  - S^T[k,q] per head via row-group-packed matmul pairs (K=Dh=64,
    tile_position (0,0)/(64,0)), both heads' scores in one PSUM tile
    [128, 1024].
  - softmax: exp via ScalarE activation (scale=1/sqrt(Dh), per-partition
    bias = -30000 on masked/padded k rows -> exact zeros), E^T in bf16.
    The scalar engine is the rate limiter of the attention inner loop
    (~1us per [128,1024] exp); everything else hides behind it.
  - O^T and the softmax denominators accumulate in ONE matmul per head
    per chunk: lhsT = [v_h | 1] (M=65) so PSUM row 64 is D_h. No
    separate denominator matmuls.
  - normalization: 1/D via reciprocal_approx_fast, per-head broadcast to
    64 partitions with K=2 bf16 selector matmuls, O^T * (1/D) on VectorE
    -> bf16, sliced into 64-q-column shards for the AllToAll.
  - the per-batch AllToAll fires right after the batch's last q-block
    is normalized; proj for batch b runs as filler during batch b+1
    (b<2) or in the tail (b=2, overlapping the split last exchange).
    Wproj-stationary proj produces out^T [1024, 256] per batch (+bias).
    Norm-path DMAs ride the gpsimd queue (batches 0-2) to dodge Sync
    head-of-line blocking, and the empty Sync queue in batch 3.

To keep the PE dense the emission order interleaves the next batch's
QKV matmuls (and the previous batch's proj) into the attention inner
loop as independent filler work. A small startup AllToAll absorbs
inter-core launch stagger off the critical path. Batch 0's K/V/Q work
is mostly woven into its own attention steps so the first scores fire
~12us in (DMA-limited on wq/wk + kb block 0 + xb q-block 0).

kernel(**inputs) accepts the full unsharded inputs and returns the full
[4, 2048, 1024] float32 output.
"""

import sys
import types

import numpy as np
import ml_dtypes

# If a caller enables BASS_TRACE without the axon NTFF profiling hook
# installed, concourse's trace path would fail importing
# antenv.axon_hooks. Provide a no-op fallback (never overrides a real
# module) so tracing degrades gracefully instead of crashing.
try:
    import antenv.axon_hooks  # noqa: F401
except ImportError:
    try:
        import antenv

        _ah = types.ModuleType("antenv.axon_hooks")
        _ah._hook = None
        _ah.set_axon_ntff_profile_hook = lambda h: setattr(_ah, "_hook", h)
        _ah.get_axon_ntff_profile_hook = lambda: _ah._hook
        sys.modules["antenv.axon_hooks"] = _ah
        antenv.axon_hooks = _ah
    except ImportError:
        pass

import concourse.bass as bass
import concourse.mybir as mybir
import concourse.tile as tile
from concourse import bacc
from concourse.bass_utils import run_bass_kernel_spmd

B = 4
N = 2048
C = 1024
H = 16
NCORES = 8
DH = C // H            # 64
HPC = H // NCORES      # 2 heads per core -> 128 channels/core
CPC = HPC * DH         # 128
ROWS = B * N           # 8192
QB = 512               # q block (one PSUM bank of f32)
QS = QB // NCORES      # 64 q columns per a2a shard
KCH = 128              # k chunk (partitions)
NQB = N // QB          # 4
CC = C // 128          # 8 contraction chunks
SCALE = DH ** -0.5     # 0.125
MASK_BIAS = -30000.0

DT = mybir.dt.float32
BF = mybir.dt.bfloat16
NPBF = ml_dtypes.bfloat16

_CACHE: dict = {}
LAST_RESULTS = None
DEBUG_DUMP = False


def _build(nkcs):
    """nkcs[b] = number of 128-row k-chunks after compaction per batch."""
    nkc = max(nkcs)          # buffer/layout stride
    nk = nkc * KCH
    nc = bacc.Bacc("TRN2", target_bir_lowering=False, debug=False, num_devices=NCORES)

    xT = nc.dram_tensor("xT", [C, ROWS], BF, kind="ExternalInput")
    xTk = nc.dram_tensor("xTk", [C, B * nk], BF, kind="ExternalInput")
    # weights arrive host-pre-rearranged to partition-major [128, CC*W] so
    # the load is 128 large contiguous descriptors instead of 1024 x 256B
    wq = nc.dram_tensor("wq", [128, CC * CPC], BF, kind="ExternalInput")
    wk = nc.dram_tensor("wk", [128, CC * CPC], BF, kind="ExternalInput")
    wv = nc.dram_tensor("wv", [128, CC * CPC], BF, kind="ExternalInput")
    wp = nc.dram_tensor("wp", [128, CC * C], BF, kind="ExternalInput")
    bvec = nc.dram_tensor("bvec", [128, CC], DT, kind="ExternalInput")
    mb = nc.dram_tensor("mb", [128, B * nkc], DT, kind="ExternalInput")
    sel2 = nc.dram_tensor("sel2", [2, 128], BF, kind="ExternalInput")
    out_ext = nc.dram_tensor("out", [C, B * 4 * QS], DT, kind="ExternalOutput")
    dbg = (
        nc.dram_tensor("dbg", [2 * (DH + 1) + 4, QB], DT, kind="ExternalOutput")
        if DEBUG_DUMP
        else None
    )
    dbg2 = (
        nc.dram_tensor("dbg2", [2 * DH, QB], DT, kind="ExternalOutput")
        if DEBUG_DUMP
        else None
    )
    dbg3 = (
        nc.dram_tensor("dbg3", [2 * DH, QB], BF, kind="ExternalOutput")
        if DEBUG_DUMP
        else None
    )
    dbg4 = (
        nc.dram_tensor("dbg4", [NCORES * 128, QS], BF, kind="ExternalOutput")
        if DEBUG_DUMP
        else None
    )
    dbg5 = (
        nc.dram_tensor("dbg5", [128, CC * NQB * QS], BF, kind="ExternalOutput")
        if DEBUG_DUMP
        else None
    )

    # k blocks for the K^T qkv matmuls (moving dim <= 512), per batch.
    # Batch 0 gets a small first block so the first S matmul only waits
    # on one k-chunk's worth of data/compute at startup.
    def mk_kblocks(n, first_small=False):
        blocks = []
        pos = 0
        if first_small and n > KCH:
            blocks.append((0, KCH))
            pos = KCH
        while pos < n:
            w = min(QB, n - pos)
            blocks.append((pos, w))
            pos += w
        return blocks

    kblocks_b = [
        mk_kblocks(nkcs[b] * KCH, first_small=(b == 0)) for b in range(B)
    ]

    with tile.TileContext(nc) as tc:
        with (
            tc.tile_pool(name="consts", bufs=1) as consts,
            tc.tile_pool(name="xpool", bufs=2) as xpool,
            tc.tile_pool(name="kpool", bufs=2) as kpool,
            tc.tile_pool(name="qkpool", bufs=2) as qkpool,
            tc.tile_pool(name="vpool", bufs=2) as vpool,
            tc.tile_pool(name="epool", bufs=6) as epool,
            tc.tile_pool(name="npool", bufs=2) as npool,
            tc.tile_pool(name="opool", bufs=2) as opool,
            tc.tile_pool(name="dram", bufs=1, space="DRAM") as dram,
            tc.tile_pool(name="s_ps", bufs=2, space="PSUM") as s_ps,
            tc.tile_pool(name="o_ps", bufs=1, space="PSUM") as o_ps,
            tc.tile_pool(name="aux_ps", bufs=2, space="PSUM") as aux_ps,
        ):
            # ---- persistent constants / weights (wp is loaded late: it is
            # first needed by proj(0) during batch 1's attention)
            wq_sb = consts.tile([128, CC, CPC], BF)
            wk_sb = consts.tile([128, CC, CPC], BF)
            wv_sb = consts.tile([128, CC, CPC], BF)
            wp_sb = consts.tile([128, CC, C], BF)
            bias_sb = consts.tile([128, CC], DT)
            mb_sb = consts.tile([128, B * nkc], DT)
            sel2_sb = consts.tile([2, 128], BF)
            nc.sync.dma_start(wq_sb[:], wq.rearrange("p (cc m) -> p cc m", cc=CC))
            nc.sync.dma_start(wk_sb[:], wk.rearrange("p (cc m) -> p cc m", cc=CC))
            nc.sync.dma_start(wv_sb[:], wv.rearrange("p (cc m) -> p cc m", cc=CC))
            nc.sync.dma_start(bias_sb[:], bvec[:])
            nc.sync.dma_start(mb_sb[:], mb[:])
            nc.sync.dma_start(sel2_sb[:], sel2[:])

            # AllToAll bounce buffers. Shard j of batch b carries q columns
            # j*64..(j+1)*64 of each q-block (so core j owns those rows).
            # Batches 0-2 exchange once per batch (bandwidth-efficient);
            # batch 3 is split [qb0-2 | qb3] so only a 64-column piece of
            # the exchange sits in the tail.
            def a2a_pair(name, cols):
                i = dram.tile([NCORES, 128, cols], BF, name=f"{name}i", tag=f"{name}i")
                o = dram.tile([NCORES, 128, cols], BF, name=f"{name}o", tag=f"{name}o")
                return i, o

            a2a = {}
            for b in range(B - 1):
                a2a[b] = a2a_pair(f"a2a{b}", NQB * QS)
            a2a["3a"] = a2a_pair("a2a3a", 3 * QS)
            a2a["3b"] = a2a_pair("a2a3b", QS)

            # startup alignment: absorb inter-core launch stagger on the
            # collective engine before real barriers sit on the critical path
            align_in = dram.tile([2, 4], BF, name="align_in", tag="align_in")
            align_out = dram.tile([2, 4], BF, name="align_out", tag="align_out")
            nc.sync.dma_start(align_in[:], sel2[0:2, 0:4])
            nc.gpsimd.collective_compute(
                "AllToAll",
                mybir.AluOpType.bypass,
                ins=[align_in.opt()],
                outs=[align_out.opt()],
                replica_groups=[list(range(NCORES))],
            )

            def emit_collective(key):
                def emit():
                    nc.gpsimd.collective_compute(
                        "AllToAll",
                        mybir.AluOpType.bypass,
                        ins=[a2a[key][0].opt()],
                        outs=[a2a[key][1].opt()],
                        replica_groups=[list(range(NCORES))],
                    )

                return emit

            xb_tiles = {}
            kb_tiles = {}
            qkv_state = {}

            def emit_xb_load(b, split_first=False):
                nk_b = nkcs[b] * KCH
                kblocks = kblocks_b[b]
                xb = xpool.tile([128, CC, N], BF, name=f"xb{b}", tag="xb")
                xs = xT[:, b * N:(b + 1) * N].rearrange("(cc p) n -> p cc n", p=128)
                kb = kpool.tile([128, CC, nk], BF, name=f"kb{b}", tag="kb")
                ks = xTk[:, b * nk:b * nk + nk_b].rearrange(
                    "(cc p) n -> p cc n", p=128
                )
                xb_tiles[b] = xb
                kb_tiles[b] = kb
                if not split_first:
                    for cc in range(CC):
                        nc.sync.dma_start(xb[:, cc, :], xs[:, cc, :])
                        nc.sync.dma_start(kb[:, cc, 0:nk_b], ks[:, cc, :])
                    return
                # batch 0: order DMAs so the first attention step's inputs
                # land first: kb block 0, xb q-block 0, rest of kb, rest of
                # xb. (Slicing matches the consumers' read regions.)
                p0, w0 = kblocks[0]
                for cc in range(CC):
                    nc.sync.dma_start(kb[:, cc, p0:p0 + w0], ks[:, cc, p0:p0 + w0])
                for cc in range(CC):
                    nc.sync.dma_start(xb[:, cc, 0:QB], xs[:, cc, 0:QB])
                for pos, w in kblocks[1:]:
                    for cc in range(CC):
                        nc.sync.dma_start(
                            kb[:, cc, pos:pos + w], ks[:, cc, pos:pos + w]
                        )
                for rb in range(1, NQB):
                    for cc in range(CC):
                        nc.sync.dma_start(
                            xb[:, cc, rb * QB:(rb + 1) * QB],
                            xs[:, cc, rb * QB:(rb + 1) * QB],
                        )

            def qkv_units(b):
                """Independent emission units for batch b's QKV (filler work).

                Returns (units, q_units, k_units, v_units) so batch 0 can
                hand-place them; for other batches `units` is spread evenly.
                """
                xb = xb_tiles[b]
                kb = kb_tiles[b]
                nkc_b = nkcs[b]
                qT = qkpool.tile([128, N], BF, name=f"qT{b}", tag="qT")
                kT = qkpool.tile([128, nk], BF, name=f"kT{b}", tag="kT")
                vt = vpool.tile([128, nkc, HPC, DH + 1], BF, name=f"vt{b}", tag="vt")
                qkv_state[b] = (qT, kT, vt)

                def ones_unit():
                    nc.vector.memset(vt[:, :, :, DH:DH + 1], 1.0)

                def q_unit(rb):
                    def emit():
                        ps = aux_ps.tile([128, QB], DT, name=f"psq{b}_{rb}", tag="aux")
                        for cc in range(CC):
                            nc.tensor.matmul(
                                ps[:],
                                wq_sb[:, cc, :],
                                xb[:, cc, rb * QB:(rb + 1) * QB],
                                start=cc == 0,
                                stop=cc == CC - 1,
                            )
                        nc.vector.tensor_copy(qT[:, rb * QB:(rb + 1) * QB], ps[:])

                    return emit

                def k_unit(pos, w):
                    def emit():
                        ps = aux_ps.tile([128, QB], DT, name=f"psk{b}_{pos}", tag="aux")
                        for cc in range(CC):
                            nc.tensor.matmul(
                                ps[:, 0:w],
                                wk_sb[:, cc, :],
                                kb[:, cc, pos:pos + w],
                                start=cc == 0,
                                stop=cc == CC - 1,
                            )
                        nc.vector.tensor_copy(kT[:, pos:pos + w], ps[:, 0:w])

                    return emit

                def v_unit(rc):
                    def emit():
                        ps = aux_ps.tile([128, QB], DT, name=f"psv{b}_{rc}", tag="aux")
                        for cc in range(CC):
                            nc.tensor.matmul(
                                ps[:, 0:CPC],
                                kb[:, cc, rc * KCH:(rc + 1) * KCH],
                                wv_sb[:, cc, :],
                                start=cc == 0,
                                stop=cc == CC - 1,
                            )
                        for h in range(HPC):
                            nc.vector.tensor_copy(
                                vt[:, rc, h, 0:DH], ps[:, h * DH:(h + 1) * DH]
                            )

                    return emit

                q_units = [q_unit(rb) for rb in range(NQB)]
                k_units = [k_unit(pos, w) for pos, w in kblocks_b[b]]
                v_units = [v_unit(rc) for rc in range(nkc_b)]
                units = [ones_unit] + q_units + k_units + v_units
                return units, q_units, k_units, v_units, ones_unit

            def attention_steps(b, carried=None):
                """One closure per (qb, kc) plus the per-qb normalization.

                `carried` is a list of closures from the previous batch
                (deferred norm_b + its collective) to weave in early.
                """
                qT, kT, vt = qkv_state[b]
                nkc_b = nkcs[b]
                kc_lists = [[] for _ in range(NQB)]
                norm_pairs = []
                for qb in range(NQB):
                    steps = kc_lists[qb]
                    o0 = o_ps.tile([DH + 1, QB], DT, name=f"o0_{b}_{qb}", tag="o0")
                    o1 = o_ps.tile([DH + 1, QB], DT, name=f"o1_{b}_{qb}", tag="o1")

                    for kc in range(nkc_b):
                        def kc_step(qb=qb, kc=kc, o0=o0, o1=o1):
                            s2 = s_ps.tile(
                                [128, 2 * QB], DT, name=f"s{b}_{qb}_{kc}", tag="s"
                            )
                            nc.tensor.matmul(
                                s2[:, 0:QB],
                                kT[0:DH, kc * KCH:(kc + 1) * KCH],
                                qT[0:DH, qb * QB:(qb + 1) * QB],
                                start=True,
                                stop=True,
                                tile_position=(0, 0),
                            )
                            nc.tensor.matmul(
                                s2[:, QB:2 * QB],
                                kT[DH:2 * DH, kc * KCH:(kc + 1) * KCH],
                                qT[DH:2 * DH, qb * QB:(qb + 1) * QB],
                                start=True,
                                stop=True,
                                tile_position=(64, 0),
                            )
                            e2 = epool.tile(
                                [128, 2 * QB], BF, name=f"e{b}_{qb}_{kc}", tag="e"
                            )
                            mcol = b * nkc + kc
                            nc.scalar.activation(
                                e2[:],
                                s2[:],
                                mybir.ActivationFunctionType.Exp,
                                bias=mb_sb[:, mcol:mcol + 1],
                                scale=SCALE,
                            )
                            st = kc == 0
                            sp = kc == nkc_b - 1
                            # O^T plus denominator in one matmul per head:
                            # vt[...,64] == 1.0 so PSUM row 64 is D_h.
                            nc.tensor.matmul(
                                o0[:],
                                vt[:, kc, 0, :],
                                e2[:, 0:QB],
                                start=st,
                                stop=sp,
                            )
                            nc.tensor.matmul(
                                o1[:],
                                vt[:, kc, 1, :],
                                e2[:, QB:2 * QB],
                                start=st,
                                stop=sp,
                            )

                        steps.append(kc_step)

                    state = {}

                    def norm_a(qb=qb, o0=o0, o1=o1, state=state):
                        # free the PSUM accumulators immediately
                        osb0 = opool.tile(
                            [DH + 1, QB], DT, name=f"osb0_{b}_{qb}", tag="osb0"
                        )
                        osb1 = opool.tile(
                            [DH + 1, QB], DT, name=f"osb1_{b}_{qb}", tag="osb1"
                        )
                        nc.vector.tensor_copy(osb0[:], o0[:])
                        nc.vector.tensor_copy(osb1[:], o1[:])
                        if DEBUG_DUMP and b == 0 and qb == 0:
                            nc.sync.dma_start(dbg[0:DH + 1, :], osb0[:])
                            nc.sync.dma_start(dbg[DH + 1:2 * (DH + 1), :], osb1[:])
                        state["osb0"] = osb0
                        state["osb1"] = osb1

                    def norm_b(qb=qb, state=state):
                        # deferred: the dd-DMA/reciprocal chain latency hides
                        # behind the next q-block's attention matmuls
                        osb0 = state["osb0"]
                        osb1 = state["osb1"]
                        dd = npool.tile([2, QB], DT, name=f"dd{b}_{qb}", tag="dd")
                        # batches 0-2: gpsimd DMA queue keeps the norm path
                        # off the Sync queue (bulk-load head-of-line
                        # blocking). Batch 3: Sync is empty (no loads, no
                        # fillers) and issues ~40% faster than gpsimd, so the
                        # tail-critical chain goes there.
                        eng = nc.sync if b == B - 1 else nc.gpsimd
                        eng.dma_start(dd[0:1, :], osb0[DH:DH + 1, :])
                        eng.dma_start(dd[1:2, :], osb1[DH:DH + 1, :])
                        dr = npool.tile([2, QB], DT, name=f"dr{b}_{qb}", tag="dr")
                        nc.vector.reciprocal_approx_fast(dr[:], dd[:])
                        drbf = npool.tile([2, QB], BF, name=f"drbf{b}_{qb}", tag="drbf")
                        nc.vector.tensor_copy(drbf[:], dr[:])
                        if DEBUG_DUMP and b == 0 and qb == 0:
                            base = 2 * (DH + 1)
                            nc.sync.dma_start(dbg[base:base + 2, :], dd[:])
                            nc.sync.dma_start(dbg[base + 2:base + 4, :], dr[:])
                        # per-head broadcast of 1/D to 64 partitions
                        drA = aux_ps.tile([DH, QB], DT, name=f"drA{b}_{qb}", tag="aux")
                        drB = aux_ps.tile([DH, QB], DT, name=f"drB{b}_{qb}", tag="aux")
                        nc.tensor.matmul(
                            drA[:], sel2_sb[:, 0:DH], drbf[:], start=True, stop=True
                        )
                        nc.tensor.matmul(
                            drB[:], sel2_sb[:, DH:2 * DH], drbf[:], start=True, stop=True
                        )
                        of0 = opool.tile([DH, QB], BF, name=f"of0_{b}_{qb}", tag="of0")
                        of1 = opool.tile([DH, QB], BF, name=f"of1_{b}_{qb}", tag="of1")
                        nc.vector.tensor_mul(of0[:], osb0[0:DH, :], drA[:])
                        nc.vector.tensor_mul(of1[:], osb1[0:DH, :], drB[:])
                        if b < B - 1:
                            dst, col = a2a[b][0], qb * QS
                        elif qb < 3:
                            dst, col = a2a["3a"][0], qb * QS
                        else:
                            dst, col = a2a["3b"][0], 0
                        eng.dma_start(
                            dst[:, 0:DH, col:col + QS].rearrange("s p j -> p s j"),
                            of0.rearrange("p (s j) -> p s j", s=NCORES),
                        )
                        eng.dma_start(
                            dst[:, DH:2 * DH, col:col + QS].rearrange("s p j -> p s j"),
                            of1.rearrange("p (s j) -> p s j", s=NCORES),
                        )

                    norm_pairs.append((norm_a, norm_b))
                # weave: kc-steps of qb, then norm_a(qb); norm_b(qb) lands
                # after the first 2 kc-steps of qb+1. A batch's collective
                # fires right after its last contributing norm_b. The tail
                # [norm_b, collective] is returned so the caller can weave
                # it into the NEXT batch (or flush it at the end).
                woven = []
                pending = list(carried) if carried else []
                for qb in range(NQB):
                    for i in range(nkc_b):
                        woven.append(kc_lists[qb][i])
                        if i == 1 and pending:
                            woven.extend(pending)
                            pending = []
                    na, nb = norm_pairs[qb]
                    woven.append(na)
                    if b == B - 1 and qb == 2:
                        # tail-critical: don't defer — the 3a exchange
                        # should fire as soon as its data exists
                        woven.append(nb)
                        woven.append(emit_collective("3a"))
                        pending = []
                        continue
                    pending = [nb]
                    if qb == NQB - 1:
                        pending.append(
                            emit_collective("3b" if b == B - 1 else b)
                        )
                return woven, pending

            def proj_units(grp, part=None):
                """Projection for batch `grp`. part=None: whole batch
                (grp < 3). For grp 3: part='a' covers q-blocks 0-2 (after
                the 3a exchange), part='b' the last 64 columns."""
                units = []
                q0, q1 = 0, NQB          # qb range this call covers
                if part == "a":
                    q1 = 3
                elif part == "b":
                    q0 = 3
                ncols = (q1 - q0) * QS

                def load_unit():
                    if part != "b":
                        ofull = qkpool.tile(
                            [128, CC, NQB, QS], BF, name=f"ofull{grp}", tag="ofull"
                        )
                        qkv_state[f"ofull{grp}"] = ofull
                    ofull = qkv_state[f"ofull{grp}"]
                    src = a2a[grp if part is None else ("3a" if part == "a" else "3b")][1]
                    nc.gpsimd.dma_start(
                        ofull[:, :, q0:q1, :],
                        src.rearrange("i p (q j) -> p i q j", q=q1 - q0),
                    )

                units.append(load_unit)

                # part 'b' (the very tail): collect all 8 oc outputs in one
                # tile and ship a single DMA — 8 serial DMA issues would sit
                # directly on the critical path.
                fo_all = [None]
                if part == "b":
                    def fo_alloc():
                        fo_all[0] = npool.tile(
                            [128, CC, ncols], DT, name=f"foall{grp}", tag="foall"
                        )
                    units.insert(0, fo_alloc)

                def oc_unit(oc):
                    def emit():
                        ofull = qkv_state[f"ofull{grp}"]
                        pps = aux_ps.tile([128, QB], DT, name=f"pp{grp}_{oc}{part or ''}", tag="aux")
                        for cc in range(CC):
                            nc.tensor.matmul(
                                pps[:, 0:ncols],
                                wp_sb[:, cc, oc * 128:(oc + 1) * 128],
                                ofull[:, cc, q0:q1, :],
                                start=cc == 0,
                                stop=cc == CC - 1,
                            )
                        if part == "b":
                            nc.vector.tensor_scalar_add(
                                fo_all[0][:, oc, :], pps[:, 0:ncols],
                                bias_sb[:, oc:oc + 1],
                            )
                            return
                        fo = npool.tile(
                            [128, ncols], DT, name=f"fo{grp}_{oc}{part or ''}", tag="fo"
                        )
                        nc.vector.tensor_scalar_add(
                            fo[:], pps[:, 0:ncols], bias_sb[:, oc:oc + 1]
                        )
                        nc.sync.dma_start(
                            out_ext[
                                oc * 128:(oc + 1) * 128,
                                grp * NQB * QS + q0 * QS:
                                grp * NQB * QS + q0 * QS + ncols,
                            ],
                            fo[:],
                        )

                    return emit

                for oc in range(CC):
                    units.append(oc_unit(oc))
                if part == "b":
                    def final_dma():
                        nc.sync.dma_start(
                            out_ext[
                                :, grp * NQB * QS + q0 * QS:
                                grp * NQB * QS + q0 * QS + ncols,
                            ].rearrange("(oc p) j -> p oc j", p=128),
                            fo_all[0][:],
                        )
                    units.append(final_dma)
                return units

            def run_interleaved(steps, fillers, pinned=None):
                """Emit `steps` in order; after step i, emit pinned[i] (a
                list) if given, and spread `fillers` evenly across steps."""
                pinned = pinned or {}
                nf = len(fillers)
                ns = len(steps)
                fi = 0
                for i, s in enumerate(steps):
                    s()
                    for p in pinned.get(i, ()):  # batch-0 hand placement
                        p()
                    if fi < nf and (i + 1) * nf >= (fi + 1) * ns:
                        fillers[fi]()
                        fi += 1
                while fi < nf:
                    fillers[fi]()
                    fi += 1

            # ---- schedule:
            #  batch 0: emit only k0/q0/v0 before attention; the rest of its
            #    QKV is pinned to the first steps. Batch 1's loads+QKV start
            #    at step 12 (after batch 0's own DMAs have drained).
            #  batch b: fillers = QKV(b+1) + proj(b-1); per-qb collectives
            #    are woven right after each norm_b.
            #  tail: last norm_b + collective(3,3), then proj(3).
            emit_xb_load(0, split_first=True)
            units0, q_units0, k_units0, v_units0, ones0 = qkv_units(0)
            ones0()
            k_units0[0]()
            q_units0[0]()
            v_units0[0]()
            nc.sync.dma_start(wp_sb[:], wp.rearrange("p (cc m) -> p cc m", cc=CC))

            pin0 = {
                0: [k_units0[i] for i in range(1, len(k_units0))] + [v_units0[1]],
                1: [v_units0[2], v_units0[3]],
                2: [v_units0[4], v_units0[5]],
                3: [v_units0[6], v_units0[7]],
                4: [v_units0[rc] for rc in range(8, nkcs[0])] + [q_units0[1]],
                6: [q_units0[2]],
                8: [q_units0[3]],
            }

            carried = None
            for b in range(B):
                fillers = []
                pinned = None
                if b == 0:
                    pinned = dict(pin0)
                    emit_xb_load(1)
                    units1 = qkv_units(1)[0]
                    # hold batch 1's QKV until batch 0's loads have drained
                    nsteps = NQB * nkcs[0]
                    for j, u in enumerate(units1):
                        pinned.setdefault(
                            12 + (j * (nsteps - 14)) // len(units1), []
                        ).append(u)
                else:
                    if b < B - 1:
                        emit_xb_load(b + 1)
                        fillers.extend(qkv_units(b + 1)[0])
                    if b < B - 1:
                        # batch 3 runs lean: proj(2) fills the tail's
                        # collective-wait gap instead of stretching the
                        # PE-bound attention phase.
                        fillers.extend(proj_units(b - 1))
                steps, carried = attention_steps(b, carried)
                run_interleaved(steps, fillers, pinned)
            # tail: last norm + tiny 3b exchange; proj(2) fills the 3a
            # collective wait, then proj(3) in two pieces so only the last
            # 64 columns depend on the final exchange.
            for u in carried:
                u()
            for u in proj_units(B - 2):
                u()
            for u in proj_units(B - 1, part="a"):
                u()
            for u in proj_units(B - 1, part="b"):
                u()

    nc.compile()
    return nc


def _prep_inputs(x, Wqkv, Wproj, bproj, mask, nkcs):
    x = np.asarray(x, dtype=np.float32)
    Wqkv = np.asarray(Wqkv, dtype=np.float32)
    Wproj = np.asarray(Wproj, dtype=np.float32)
    bproj = np.asarray(bproj, dtype=np.float32)
    mask = np.asarray(mask)
    nkc = max(nkcs)
    nk = nkc * KCH

    x2 = x.reshape(ROWS, C)
    xT = np.ascontiguousarray(x2.T).astype(NPBF)
    # compacted K/V tokens: unmasked columns per batch, zero-padded to nk
    xTk = np.zeros((C, B * nk), dtype=NPBF)
    mbias = np.full((B, nk), np.float32(MASK_BIAS), dtype=np.float32)
    for b in range(B):
        idx = np.nonzero(mask[b] == 0)[0]
        cnt = len(idx)
        xTk[:, b * nk: b * nk + cnt] = xT[:, b * N + idx]
        mbias[b, :cnt] = 0.0
    mb_arr = np.ascontiguousarray(
        mbias.reshape(B, nkc, 128).transpose(2, 0, 1).reshape(128, B * nkc)
    ).astype(np.float32)

    def pmajor(w):
        # [C, W] -> [128, CC*W]: row cc*128+p becomes partition p, block cc
        W = w.shape[1]
        return np.ascontiguousarray(
            w.reshape(CC, 128, W).transpose(1, 0, 2).reshape(128, CC * W)
        ).astype(NPBF)

    wp_bf = pmajor(Wproj)
    bias_r = np.ascontiguousarray(bproj.reshape(CC, 128).T).astype(np.float32)
    sel2 = np.zeros((2, 128), np.float32)
    sel2[0, 0:64] = 1.0
    sel2[1, 64:128] = 1.0
    sel2 = sel2.astype(NPBF)

    in_maps = []
    for c in range(NCORES):
        cols = slice(c * CPC, (c + 1) * CPC)
        in_maps.append(
            dict(
                xT=xT,
                xTk=xTk,
                wq=pmajor(Wqkv[:, cols]),
                wk=pmajor(Wqkv[:, C:][:, cols]),
                wv=pmajor(Wqkv[:, 2 * C:][:, cols]),
                wp=wp_bf,
                bvec=bias_r,
                mb=mb_arr,
                sel2=sel2,
            )
        )
    return in_maps


def kernel(x, Wqkv, Wproj, bproj, mask):
    global LAST_RESULTS
    mask = np.asarray(mask)
    counts = (mask == 0).sum(axis=1)
    nkcs = tuple(max(1, -(-int(c) // KCH)) for c in counts)
    if nkcs not in _CACHE:
        _CACHE[nkcs] = _build(nkcs)
    nc = _CACHE[nkcs]
    in_maps = _prep_inputs(x, Wqkv, Wproj, bproj, mask, nkcs)
    res = run_bass_kernel_spmd(nc, in_maps, list(range(NCORES)))
    LAST_RESULTS = res
    out = np.empty((ROWS, C), dtype=np.float32)
    for c in range(NCORES):
        oT = res.results[c]["out"]  # [1024 oc, B * 4 qb * 64 q] = final^T
        for b in range(B):
            for qb in range(NQB):
                rows = slice(
                    b * N + qb * QB + c * QS, b * N + qb * QB + (c + 1) * QS
                )
                out[rows, :] = oT[:, b * NQB * QS + qb * QS:
                                  b * NQB * QS + (qb + 1) * QS].T
    return out.reshape(B, N, C)
